# revision 13
# baseline (speedup 1.0000x reference)
"""Trainium2 Bass kernel for nn_Attention_KV (dense transformer attention
with K=Q sharing and a linear positional bias), distributed over 8 cores.

Sharding: data-parallel over batch (core c owns batch element c; B == 8).
The positional bias pos_bias(i,j) = sum_p pos[0,i,j,p] * w_pos[p] is
head- and batch-independent, so its computation is row-sharded over the 8
cores (core c computes the 128 j-rows of the *transposed* bias it owns)
and AllGathered (in two i-halves, so attention can start on the first
half while the second is still in flight).

All attention math is done with scores kept TRANSPOSED (j on partitions,
i on the free axis). Because dots = k @ k^T is symmetric this costs
nothing, and it makes softmax + the attn @ v contraction expressible
without any on-chip transpose:
  - scores^T lands directly in PSUM: dots matmuls + an identity matmul
    that adds pos_bias^T/c (pos is pre-divided by c = scale*sum(w_pos)
    on-device, so exp(scale=c) on the Scalar engine applies both the
    dot-product scaling and the bias in one pass)
  - attn@v as lhsT = v_ext (with a ones column appended -> row 64 of the
    result is the softmax denominator Z), rhs = exp(scores^T)
  - normalization folded into the PSUM->SBUF copy of U
b_pos (a scalar added to every score) is dropped: softmax is shift
invariant.
"""

import sys

sys.path.insert(0, "/opt/trn_rl_repo")

import numpy as np

import concourse.bacc as bacc
import concourse.bass as bass
import concourse.mybir as mybir
from concourse import tile
from concourse.bass_utils import run_bass_kernel_spmd

B, N, DIM, H, POS_DIM = 8, 1024, 512, 8, 50
D = DIM // H  # 64
NC = 8  # cores
JT = N // 128  # 8 j-tiles
SCALE = float(DIM) ** -0.5

F32 = mybir.dt.float32
BF16 = mybir.dt.bfloat16
F32R = mybir.dt.float32r
AX = mybir.AxisListType
ALU = mybir.AluOpType
ACTF = mybir.ActivationFunctionType

POS_CHUNK = 64  # i-rows of pos processed per DVE reduce


def build_program(reps: int = 1):
    nc = bacc.Bacc("TRN2", target_bir_lowering=False, debug=False)

    # ---- DRAM parameters (per-core) ----
    xT_d = nc.declare_dram_parameter("xT", [DIM, N], F32R, isOutput=False)
    wkvT_d = nc.declare_dram_parameter("wkvT", [DIM, 2 * DIM], F32R, isOutput=False)
    wout_d = nc.declare_dram_parameter("wout", [DIM, DIM], F32R, isOutput=False)
    bout_d = nc.declare_dram_parameter("bout", [1, DIM], F32R, isOutput=False)
    wposr_d = nc.declare_dram_parameter(
        "wposr", [128, POS_CHUNK, POS_DIM], BF16, isOutput=False
    )
    posT_d = nc.declare_dram_parameter("posT", [128, N, POS_DIM], BF16, isOutput=False)
    ones_d = nc.declare_dram_parameter("ones", [128, 128], F32R, isOutput=False)
    id_d = nc.declare_dram_parameter("idm", [128, 128], BF16, isOutput=False)
    y_d = nc.declare_dram_parameter("y", [N, DIM], F32, isOutput=True)

    with tile.TileContext(nc) as tc:
        with (
            tc.tile_pool(name="persist", bufs=1) as pp,
            tc.tile_pool(name="pos_in", bufs=2) as pos_pool,
            tc.tile_pool(name="exps", bufs=4) as epool,
            tc.tile_pool(name="outsb", bufs=2) as opool,
            tc.tile_pool(name="mm_ps", bufs=2, space="PSUM") as mmps,
            tc.tile_pool(name="dots_ps", bufs=4, space="PSUM") as dotsps,
            tc.tile_pool(name="up_ps", bufs=2, space="PSUM") as upps,
            tc.tile_pool(name="dram", bufs=1, space="DRAM") as dram,
        ):
            for _rep in range(reps):
                # ---- preload small tensors ----
                wposr = pp.tile([128, POS_CHUNK, POS_DIM], BF16, tag="wposr")
                nc.sync.dma_start(wposr[:], wposr_d[:])
                ones1 = pp.tile([1, 128], F32R, tag="ones1")
                nc.sync.dma_start(ones1[:], ones_d[0:1, :])
                idm = pp.tile([128, 128], BF16, tag="idm")
                nc.sync.dma_start(idm[:], id_d[:])

                # c = scale * sum(w_pos) on every partition; wposr /= c so the
                # pos-bias accumulates pre-divided and exp(scale=c) restores it.
                c_ap = pp.tile([128, 1], F32, tag="c_ap")
                ic_ap = pp.tile([128, 1], F32, tag="ic_ap")
                nc.vector.tensor_reduce(c_ap[:], wposr[:, 0, :], axis=AX.X, op=ALU.add)
                nc.scalar.mul(c_ap[:], c_ap[:], SCALE)
                nc.vector.reciprocal(ic_ap[:], c_ap[:])
                with nc.allow_low_precision(reason="w_pos/c in bf16 is intended"):
                    nc.vector.tensor_scalar_mul(wposr[:], wposr[:], ic_ap[:])

                # ---- preload weights / x ----
                wkvT = [
                    pp.tile([128, 2 * DIM], F32R, name=f"wkvT{t}", tag=f"wkvT{t}")
                    for t in range(4)
                ]
                xT = [
                    pp.tile([128, N], F32R, name=f"xT{t}", tag=f"xT{t}")
                    for t in range(4)
                ]
                for t in range(4):
                    nc.sync.dma_start(wkvT[t][:], wkvT_d[t * 128 : (t + 1) * 128, :])
                    nc.sync.dma_start(xT[t][:], xT_d[t * 128 : (t + 1) * 128, :])
                wout = [
                    pp.tile([64, DIM], F32R, name=f"wout{h}", tag=f"wout{h}")
                    for h in range(H)
                ]
                for h in range(H):
                    nc.sync.dma_start(wout[h][:], wout_d[h * 64 : (h + 1) * 64, :])
                bout = pp.tile([1, DIM], F32R, tag="bout")
                nc.sync.dma_start(bout[:], bout_d[:])

                # ---- pos-bias phase: this core's 128 j-rows, all i ----
                pbm = pp.tile([128, N], BF16, tag="pbm")
                NQ = 4
                pb_in = [
                    dram.tile([128, N // NQ], BF16, name=f"pbin{x}") for x in range(NQ)
                ]
                pb_all = [
                    dram.tile([N, N // NQ], BF16, name=f"pball{x}") for x in range(NQ)
                ]
                nch = N // POS_CHUNK
                for ic in range(nch):
                    sl = slice(ic * POS_CHUNK, (ic + 1) * POS_CHUNK)
                    pt = pos_pool.tile(
                        [128, POS_CHUNK, POS_DIM], BF16, name="pchunk", tag="pchunk"
                    )
                    nc.sync.dma_start(pt[:], posT_d[:, sl, :])
                    nc.vector.tensor_tensor(pt[:], pt[:], wposr[:], op=ALU.mult)
                    with nc.allow_low_precision(
                        reason="f32r keeps full fp32 bits; rounding happens in PE"
                    ):
                        nc.vector.tensor_reduce(
                            pbm[:, sl], pt[:], axis=AX.X, op=ALU.add
                        )
                    if (ic + 1) % (nch // NQ) == 0:
                        q = (ic + 1) // (nch // NQ) - 1
                        qsl = slice(q * (N // NQ), (q + 1) * (N // NQ))
                        nc.sync.dma_start(pb_in[q][:], pbm[:, qsl])
                        nc.gpsimd.collective_compute(
                            "AllGather",
                            ALU.bypass,
                            replica_groups=[list(range(NC))],
                            ins=[pb_in[q].opt()],
                            outs=[pb_all[q].opt()],
                        )

                # ---- kv phase: kT (k transposed) and v_ext (v + ones col) ----
                kT = [
                    pp.tile([128, N], F32R, name=f"kT{t}", tag=f"kT{t}")
                    for t in range(4)
                ]
                for t in range(4):
                    for nchunk in range(2):
                        ps = mmps.tile([128, 512], F32, name="mmtile", tag="mm")
                        for dc in range(4):
                            nc.tensor.matmul(
                                ps[:],
                                wkvT[dc][:, t * 128 : (t + 1) * 128],
                                xT[dc][:, nchunk * 512 : (nchunk + 1) * 512],
                                start=(dc == 0),
                                stop=(dc == 3),
                            )
                        nc.vector.tensor_copy(
                            kT[t][:, nchunk * 512 : (nchunk + 1) * 512], ps[:]
                        )

                vext = [
                    pp.tile([128, H, D + 1], F32R, name=f"vext{t}", tag=f"vext{t}")
                    for t in range(JT)
                ]
                for nt in range(JT):
                    ps = mmps.tile([128, 512], F32, name="mmtile", tag="mm")
                    for dc in range(4):
                        nc.tensor.matmul(
                            ps[:],
                            xT[dc][:, nt * 128 : (nt + 1) * 128],
                            wkvT[dc][:, DIM : 2 * DIM],
                            start=(dc == 0),
                            stop=(dc == 3),
                        )
                    nc.sync.dma_start(vext[nt][:, :, D : D + 1], ones_d[:, 0:H])
                    nc.vector.tensor_copy(
                        vext[nt][:, :, 0:D],
                        ps[:].rearrange("p (h d) -> p h d", h=H),
                    )

                # ---- gathered pos-bias^T tiles (pre-divided by c) ----
                posT_sb = [
                    pp.tile([128, N], BF16, name=f"posT{j}", tag=f"posT{j}")
                    for j in range(JT)
                ]
                for j in range(JT):
                    for q in range(4):
                        qsl = slice(q * (N // 4), (q + 1) * (N // 4))
                        nc.sync.dma_start(
                            posT_sb[j][:, qsl],
                            pb_all[q][j * 128 : (j + 1) * 128, :],
                        )

                # ---- attention, i-half outer so half 0 runs while the
                # second AllGather is still in flight ----
                UT = [
                    pp.tile([64, N], F32R, name=f"UT{h}", tag=f"UT{h}")
                    for h in range(H)
                ]
                rzb = pp.tile([64, N // 2], F32, tag="rzb")
                rzrow = pp.tile([65, 512], F32, tag="rzrow")
                rz_bounce = dram.tile([1, N], F32)
                for icn in range(2):
                    isl = slice(icn * 512, (icn + 1) * 512)
                    for h in range(H):
                        kt = kT[h // 2]
                        pr = slice(64 * (h % 2), 64 * (h % 2) + 64)
                        up = upps.tile([D + 1, 512], F32, name="uptile", tag="up")
                        for jt in range(JT):
                            dots = dotsps.tile(
                                [128, 512], F32, name="dotstile", tag="dots"
                            )
                            nc.tensor.matmul(
                                dots[:],
                                kt[pr, jt * 128 : (jt + 1) * 128],
                                kt[pr, isl],
                                start=True,
                                stop=False,
                            )
                            nc.tensor.matmul(
                                dots[:],
                                idm[:],
                                posT_sb[jt][:, isl],
                                start=False,
                                stop=True,
                            )
                            es = epool.tile([128, 512], F32R, name="expS", tag="expS")
                            nc.scalar.activation(
                                es[:], dots[:], ACTF.Exp, scale=c_ap[:]
                            )
                            nc.tensor.matmul(
                                up[:],
                                vext[jt][:, h, :],
                                es[:],
                                start=(jt == 0),
                                stop=(jt == JT - 1),
                            )
                        # row 64 of up = Z; reciprocal staged through row 64 of
                        # pbm (free after the pos phase), broadcast via DRAM.
                        nc.vector.reciprocal(rzrow[64:65, :], up[64:65, :])
                        nc.sync.dma_start(rz_bounce[0:1, isl], rzrow[64:65, :])
                        nc.sync.dma_start(
                            rzb[:], rz_bounce[0:1, isl].to_broadcast([64, N // 2])
                        )
                        nc.vector.tensor_tensor(
                            UT[h][:, isl], up[0:64, :], rzb[:], op=ALU.mult
                        )

                # ---- final projection: y = U^T.T @ Wout^T + b_out ----
                for it in range(JT):
                    isl = slice(it * 128, (it + 1) * 128)
                    fps = mmps.tile([128, 512], F32, name="mmtile", tag="mm")
                    for h in range(H):
                        nc.tensor.matmul(
                            fps[:],
                            UT[h][:, isl],
                            wout[h][:],
                            start=(h == 0),
                            stop=False,
                        )
                    nc.tensor.matmul(fps[:], ones1[:], bout[:], start=False, stop=True)
                    ot = opool.tile([128, 512], F32, name="osb", tag="osb")
                    nc.vector.tensor_copy(ot[:], fps[:])
                    nc.sync.dma_start(y_d[isl, :], ot[:])

    nc.compile()
    return nc


_CACHE = {}


def _get_program():
    if "nc" not in _CACHE:
        _CACHE["nc"] = build_program()
    return _CACHE["nc"]


def _host_shard(x, pos, W_kv, W_out, b_out, w_pos, b_pos):
    """Build the 8 per-core input maps (pure layout work, no math)."""
    x = np.asarray(x, dtype=np.float32)
    pos = np.asarray(pos, dtype=np.float32)
    W_kv = np.asarray(W_kv, dtype=np.float32)
    W_out = np.asarray(W_out, dtype=np.float32)
    b_out = np.asarray(b_out, dtype=np.float32)
    w_pos = np.asarray(w_pos, dtype=np.float32)

    wkvT = np.ascontiguousarray(W_kv.T)  # (512, 1024)
    wout = np.ascontiguousarray(W_out.T)  # (512, 512)
    boutr = b_out.reshape(1, DIM)
    import ml_dtypes
    wposr = np.ascontiguousarray(
        np.broadcast_to(w_pos.astype(ml_dtypes.bfloat16), (128, POS_CHUNK, POS_DIM))
    )
    ones_arr = np.ones((128, 128), dtype=np.float32)
    id_arr = np.eye(128, dtype=ml_dtypes.bfloat16)

    in_maps = []
    for c in range(NC):
        xT = np.ascontiguousarray(x[c].T)  # (512, 1024)
        posT = np.ascontiguousarray(
            pos[0, :, c * 128 : (c + 1) * 128, :]
            .transpose(1, 0, 2)
            .astype(ml_dtypes.bfloat16)
        )  # (128 j, 1024 i, 50) bf16
        in_maps.append(
            {
                "xT": xT,
                "wkvT": wkvT,
                "wout": wout,
                "bout": boutr,
                "wposr": wposr,
                "posT": posT,
                "ones": ones_arr,
                "idm": id_arr,
            }
        )
    return in_maps


def kernel(**inputs) -> np.ndarray:
    nc = _get_program()
    in_maps = _host_shard(**inputs)
    res = run_bass_kernel_spmd(nc, in_maps, list(range(NC)))
    out = np.stack([res.results[c]["y"] for c in range(NC)], axis=0)
    return out.astype(np.float32)


if __name__ == "__main__":
    import reference

    inputs = {k: np.asarray(v) for k, v in reference.setup_inputs().items()}
    expected = np.asarray(reference.reference(**inputs))
    actual = kernel(**inputs)
    err = np.abs(actual - expected).max()
    rel = err / np.abs(expected).max()
    print(f"absmax err: {err:.3e}  rel: {rel:.3e}")


# revision 14
# speedup vs baseline: 1.2987x; 1.2987x over previous
"""Trainium2 Bass kernel for nn_Attention_KV (dense transformer attention
with K=Q sharing and a linear positional bias), distributed over 8 cores.

Sharding: data-parallel over batch (core c owns batch element c; B == 8).
The positional bias pos_bias(i,j) = sum_p pos[0,i,j,p] * w_pos[p] is
head- and batch-independent, so its computation is row-sharded over the 8
cores (core c computes the 128 j-rows of the *transposed* bias it owns)
and AllGathered (in two i-halves, so attention can start on the first
half while the second is still in flight).

All attention math is done with scores kept TRANSPOSED (j on partitions,
i on the free axis). Because dots = k @ k^T is symmetric this costs
nothing, and it makes softmax + the attn @ v contraction expressible
without any on-chip transpose:
  - scores^T lands directly in PSUM: dots matmuls + an identity matmul
    that adds pos_bias^T/c (pos is pre-divided by c = scale*sum(w_pos)
    on-device, so exp(scale=c) on the Scalar engine applies both the
    dot-product scaling and the bias in one pass)
  - attn@v as lhsT = v_ext (with a ones column appended -> row 64 of the
    result is the softmax denominator Z), rhs = exp(scores^T)
  - normalization folded into the PSUM->SBUF copy of U
b_pos (a scalar added to every score) is dropped: softmax is shift
invariant.
"""

import sys

sys.path.insert(0, "/opt/trn_rl_repo")

import numpy as np

import concourse.bacc as bacc
import concourse.bass as bass
import concourse.mybir as mybir
from concourse import tile
from concourse.bass_utils import run_bass_kernel_spmd

B, N, DIM, H, POS_DIM = 8, 1024, 512, 8, 50
D = DIM // H  # 64
NC = 8  # cores
JT = N // 128  # 8 j-tiles
SCALE = float(DIM) ** -0.5

F32 = mybir.dt.float32
BF16 = mybir.dt.bfloat16
F32R = mybir.dt.float32r
AX = mybir.AxisListType
ALU = mybir.AluOpType
ACTF = mybir.ActivationFunctionType

POS_CHUNK = 64  # i-rows of pos processed per DVE reduce


def build_program(reps: int = 1, skip_collective: bool = False):
    nc = bacc.Bacc("TRN2", target_bir_lowering=False, debug=False)

    # ---- DRAM parameters (per-core) ----
    xT_d = nc.declare_dram_parameter("xT", [DIM, N], F32R, isOutput=False)
    wkvT_d = nc.declare_dram_parameter("wkvT", [DIM, 2 * DIM], F32R, isOutput=False)
    wout_d = nc.declare_dram_parameter("wout", [DIM, DIM], F32R, isOutput=False)
    bout_d = nc.declare_dram_parameter("bout", [1, DIM], F32R, isOutput=False)
    wposr_d = nc.declare_dram_parameter(
        "wposr", [128, POS_CHUNK, POS_DIM], BF16, isOutput=False
    )
    posT_d = nc.declare_dram_parameter("posT", [128, N, POS_DIM], BF16, isOutput=False)
    ones_d = nc.declare_dram_parameter("ones", [128, 128], F32R, isOutput=False)
    id_d = nc.declare_dram_parameter("idm", [128, 128], BF16, isOutput=False)
    y_d = nc.declare_dram_parameter("y", [N, DIM], F32, isOutput=True)

    with tile.TileContext(nc) as tc:
        with (
            tc.tile_pool(name="persist", bufs=1) as pp,
            tc.tile_pool(name="pos_in", bufs=2) as pos_pool,
            tc.tile_pool(name="exps", bufs=4) as epool,
            tc.tile_pool(name="outsb", bufs=2) as opool,
            tc.tile_pool(name="mm_ps", bufs=2, space="PSUM") as mmps,
            tc.tile_pool(name="dots_ps", bufs=4, space="PSUM") as dotsps,
            tc.tile_pool(name="up_ps", bufs=2, space="PSUM") as upps,
            tc.tile_pool(name="dram", bufs=1, space="DRAM") as dram,
        ):
            for _rep in range(reps):
                # ---- preload small tensors ----
                wposr = pp.tile([128, POS_CHUNK, POS_DIM], BF16, tag="wposr")
                nc.sync.dma_start(wposr[:], wposr_d[:])
                ones1 = pp.tile([1, 128], F32R, tag="ones1")
                nc.sync.dma_start(ones1[:], ones_d[0:1, :])
                idm = pp.tile([128, 128], BF16, tag="idm")
                nc.sync.dma_start(idm[:], id_d[:])

                # c = scale * sum(w_pos) on every partition; wposr /= c so the
                # pos-bias accumulates pre-divided and exp(scale=c) restores it.
                c_ap = pp.tile([128, 1], F32, tag="c_ap")
                ic_ap = pp.tile([128, 1], F32, tag="ic_ap")
                nc.vector.tensor_reduce(c_ap[:], wposr[:, 0, :], axis=AX.X, op=ALU.add)
                nc.scalar.mul(c_ap[:], c_ap[:], SCALE)
                nc.vector.reciprocal(ic_ap[:], c_ap[:])
                with nc.allow_low_precision(reason="w_pos/c in bf16 is intended"):
                    nc.vector.tensor_scalar_mul(wposr[:], wposr[:], ic_ap[:])

                # ---- preload weights / x ----
                wkvT = [
                    pp.tile([128, 2 * DIM], F32R, name=f"wkvT{t}", tag=f"wkvT{t}")
                    for t in range(4)
                ]
                xT = [
                    pp.tile([128, N], F32R, name=f"xT{t}", tag=f"xT{t}")
                    for t in range(4)
                ]
                for t in range(4):
                    nc.sync.dma_start(wkvT[t][:], wkvT_d[t * 128 : (t + 1) * 128, :])
                    nc.sync.dma_start(xT[t][:], xT_d[t * 128 : (t + 1) * 128, :])
                wout = [
                    pp.tile([64, DIM], F32R, name=f"wout{h}", tag=f"wout{h}")
                    for h in range(H)
                ]
                for h in range(H):
                    nc.sync.dma_start(wout[h][:], wout_d[h * 64 : (h + 1) * 64, :])
                bout = pp.tile([1, DIM], F32R, tag="bout")
                nc.sync.dma_start(bout[:], bout_d[:])

                # ---- pos-bias phase: this core's 128 j-rows, all i ----
                pbm = pp.tile([128, N], BF16, tag="pbm")
                NQ = 4
                pb_in = [
                    dram.tile([128, N // NQ], BF16, name=f"pbin{x}") for x in range(NQ)
                ]
                pb_all = [
                    dram.tile([N, N // NQ], BF16, name=f"pball{x}") for x in range(NQ)
                ]
                nch = N // POS_CHUNK
                for ic in range(nch):
                    sl = slice(ic * POS_CHUNK, (ic + 1) * POS_CHUNK)
                    pt = pos_pool.tile(
                        [128, POS_CHUNK, POS_DIM], BF16, name="pchunk", tag="pchunk"
                    )
                    nc.sync.dma_start(pt[:], posT_d[:, sl, :])
                    nc.vector.tensor_tensor(pt[:], pt[:], wposr[:], op=ALU.mult)
                    with nc.allow_low_precision(
                        reason="f32r keeps full fp32 bits; rounding happens in PE"
                    ):
                        nc.vector.tensor_reduce(
                            pbm[:, sl], pt[:], axis=AX.X, op=ALU.add
                        )
                    if (ic + 1) % (nch // NQ) == 0:
                        q = (ic + 1) // (nch // NQ) - 1
                        qsl = slice(q * (N // NQ), (q + 1) * (N // NQ))
                        nc.sync.dma_start(pb_in[q][:], pbm[:, qsl])
                        if not skip_collective:
                            nc.gpsimd.collective_compute(
                                "AllGather",
                                ALU.bypass,
                                replica_groups=[list(range(NC))],
                                ins=[pb_in[q].opt()],
                                outs=[pb_all[q].opt()],
                            )

                # ---- kv phase: kT (k transposed) and v_ext (v + ones col) ----
                kT = [
                    pp.tile([128, N], F32R, name=f"kT{t}", tag=f"kT{t}")
                    for t in range(4)
                ]
                for t in range(4):
                    for nchunk in range(2):
                        ps = mmps.tile([128, 512], F32, name="mmtile", tag="mm")
                        for dc in range(4):
                            nc.tensor.matmul(
                                ps[:],
                                wkvT[dc][:, t * 128 : (t + 1) * 128],
                                xT[dc][:, nchunk * 512 : (nchunk + 1) * 512],
                                start=(dc == 0),
                                stop=(dc == 3),
                            )
                        nc.vector.tensor_copy(
                            kT[t][:, nchunk * 512 : (nchunk + 1) * 512], ps[:]
                        )

                vext = [
                    pp.tile([128, H, D + 1], F32R, name=f"vext{t}", tag=f"vext{t}")
                    for t in range(JT)
                ]
                for nt in range(JT):
                    ps = mmps.tile([128, 512], F32, name="mmtile", tag="mm")
                    for dc in range(4):
                        nc.tensor.matmul(
                            ps[:],
                            xT[dc][:, nt * 128 : (nt + 1) * 128],
                            wkvT[dc][:, DIM : 2 * DIM],
                            start=(dc == 0),
                            stop=(dc == 3),
                        )
                    nc.sync.dma_start(vext[nt][:, :, D : D + 1], ones_d[:, 0:H])
                    nc.vector.tensor_copy(
                        vext[nt][:, :, 0:D],
                        ps[:].rearrange("p (h d) -> p h d", h=H),
                    )

                # ---- gathered pos-bias^T tiles (pre-divided by c) ----
                posT_sb = [
                    pp.tile([128, N], BF16, name=f"posT{j}", tag=f"posT{j}")
                    for j in range(JT)
                ]
                for j in range(JT):
                    for q in range(4):
                        qsl = slice(q * (N // 4), (q + 1) * (N // 4))
                        nc.sync.dma_start(
                            posT_sb[j][:, qsl],
                            pb_all[q][j * 128 : (j + 1) * 128, :],
                        )

                # ---- attention, i-half outer so half 0 runs while the
                # second AllGather is still in flight ----
                UT = [
                    pp.tile([64, N], F32R, name=f"UT{h}", tag=f"UT{h}")
                    for h in range(H)
                ]
                rzb = pp.tile([64, N // 2], F32, tag="rzb")
                rzrow = pp.tile([65, 512], F32, tag="rzrow")
                rz_bounce = dram.tile([1, N], F32)
                for icn in range(2):
                    isl = slice(icn * 512, (icn + 1) * 512)
                    for h in range(H):
                        kt = kT[h // 2]
                        pr = slice(64 * (h % 2), 64 * (h % 2) + 64)
                        up = upps.tile([D + 1, 512], F32, name="uptile", tag="up")
                        for jt in range(JT):
                            dots = dotsps.tile(
                                [128, 512], F32, name="dotstile", tag="dots"
                            )
                            nc.tensor.matmul(
                                dots[:],
                                kt[pr, jt * 128 : (jt + 1) * 128],
                                kt[pr, isl],
                                start=True,
                                stop=False,
                            )
                            nc.tensor.matmul(
                                dots[:],
                                idm[:],
                                posT_sb[jt][:, isl],
                                start=False,
                                stop=True,
                            )
                            es = epool.tile([128, 512], F32R, name="expS", tag="expS")
                            nc.scalar.activation(
                                es[:], dots[:], ACTF.Exp, scale=c_ap[:]
                            )
                            nc.tensor.matmul(
                                up[:],
                                vext[jt][:, h, :],
                                es[:],
                                start=(jt == 0),
                                stop=(jt == JT - 1),
                            )
                        # row 64 of up = Z; reciprocal staged through row 64 of
                        # pbm (free after the pos phase), broadcast via DRAM.
                        nc.vector.reciprocal(rzrow[64:65, :], up[64:65, :])
                        nc.sync.dma_start(rz_bounce[0:1, isl], rzrow[64:65, :])
                        nc.sync.dma_start(
                            rzb[:], rz_bounce[0:1, isl].to_broadcast([64, N // 2])
                        )
                        nc.vector.tensor_tensor(
                            UT[h][:, isl], up[0:64, :], rzb[:], op=ALU.mult
                        )

                # ---- final projection: y = U^T.T @ Wout^T + b_out ----
                for it in range(JT):
                    isl = slice(it * 128, (it + 1) * 128)
                    fps = mmps.tile([128, 512], F32, name="mmtile", tag="mm")
                    for h in range(H):
                        nc.tensor.matmul(
                            fps[:],
                            UT[h][:, isl],
                            wout[h][:],
                            start=(h == 0),
                            stop=False,
                        )
                    nc.tensor.matmul(fps[:], ones1[:], bout[:], start=False, stop=True)
                    ot = opool.tile([128, 512], F32, name="osb", tag="osb")
                    nc.vector.tensor_copy(ot[:], fps[:])
                    nc.sync.dma_start(y_d[isl, :], ot[:])

    nc.compile()
    return nc


_CACHE = {}


def _get_program():
    if "nc" not in _CACHE:
        _CACHE["nc"] = build_program()
    return _CACHE["nc"]


def _host_shard(x, pos, W_kv, W_out, b_out, w_pos, b_pos):
    """Build the 8 per-core input maps (pure layout work, no math)."""
    x = np.asarray(x, dtype=np.float32)
    pos = np.asarray(pos, dtype=np.float32)
    W_kv = np.asarray(W_kv, dtype=np.float32)
    W_out = np.asarray(W_out, dtype=np.float32)
    b_out = np.asarray(b_out, dtype=np.float32)
    w_pos = np.asarray(w_pos, dtype=np.float32)

    wkvT = np.ascontiguousarray(W_kv.T)  # (512, 1024)
    wout = np.ascontiguousarray(W_out.T)  # (512, 512)
    boutr = b_out.reshape(1, DIM)
    import ml_dtypes
    wposr = np.ascontiguousarray(
        np.broadcast_to(w_pos.astype(ml_dtypes.bfloat16), (128, POS_CHUNK, POS_DIM))
    )
    ones_arr = np.ones((128, 128), dtype=np.float32)
    id_arr = np.eye(128, dtype=ml_dtypes.bfloat16)

    in_maps = []
    for c in range(NC):
        xT = np.ascontiguousarray(x[c].T)  # (512, 1024)
        posT = np.ascontiguousarray(
            pos[0, :, c * 128 : (c + 1) * 128, :]
            .transpose(1, 0, 2)
            .astype(ml_dtypes.bfloat16)
        )  # (128 j, 1024 i, 50) bf16
        in_maps.append(
            {
                "xT": xT,
                "wkvT": wkvT,
                "wout": wout,
                "bout": boutr,
                "wposr": wposr,
                "posT": posT,
                "ones": ones_arr,
                "idm": id_arr,
            }
        )
    return in_maps


def kernel(**inputs) -> np.ndarray:
    nc = _get_program()
    in_maps = _host_shard(**inputs)
    res = run_bass_kernel_spmd(nc, in_maps, list(range(NC)))
    out = np.stack([res.results[c]["y"] for c in range(NC)], axis=0)
    return out.astype(np.float32)


if __name__ == "__main__":
    import reference

    inputs = {k: np.asarray(v) for k, v in reference.setup_inputs().items()}
    expected = np.asarray(reference.reference(**inputs))
    actual = kernel(**inputs)
    err = np.abs(actual - expected).max()
    rel = err / np.abs(expected).max()
    print(f"absmax err: {err:.3e}  rel: {rel:.3e}")
